# revision 14
# baseline (speedup 1.0000x reference)
"""Trainium2 Bass kernel for the GINAR cell (gnn_message_passing).

Strategy: data-parallel over batch B=8 across 8 NeuronCores (1 batch element
per core).  All tensors are kept feature-major ([E, N] / [2E, N]) on-chip;
contraction-side operands are produced in i-major layout ([N, E]-tiles) via
cheap PE transposes.  The symmetric-normalized Laplacians L = d * M^T * d are
never materialized: the host uploads graph_data pre-transposed (pure layout),
the d row-sum vector is computed on-device by a 8-way-sharded ones-matmul
pre-pass + AllReduce, and the two diagonal scalings are folded into adjacent
matmul operands (per-partition tensor_scalar ops).

Attention softmax needs no max-subtraction (scores <= ~8.1 for this model
family); the graph-learn softmax uses a full max-subtracted stable softmax.
The `where(adj > 0, e, -inf)` mask in the reference is a provable no-op
(softmax output is strictly positive, so adj > 0 everywhere).
"""

import numpy as np

import concourse.bass as bass
import concourse.mybir as mybir
import concourse.tile as tile
from concourse import bacc
from concourse.bass_utils import run_bass_kernel_spmd
from concourse.masks import make_identity

F32 = mybir.dt.float32
AF = mybir.ActivationFunctionType
ALU = mybir.AluOpType
AX = mybir.AxisListType

B, N, IN, E, G = 8, 2048, 12, 64, 32
P = 128
NB = N // P          # 16 row-blocks
FC = 512             # fp32 matmul moving-operand max free dim
NCH = N // FC        # 4 chunks
D2 = 2 * E           # 128
NCORES = 8
SHARD = N // NCORES  # 256 rows of the d-prepass per core

_CACHE = {}


def _ap(handle_ap, offset, pattern):
    return bass.AP(tensor=handle_ap.tensor, offset=handle_ap.offset + offset,
                   ap=[list(p) for p in pattern])


def build_program(sharded_d=True):
    nc = bacc.Bacc("TRN2", target_bir_lowering=False, debug=False,
                   num_devices=NCORES if sharded_d else 1)

    di = {}
    def inp(name, shape):
        di[name] = nc.dram_tensor(name, shape, F32, kind="ExternalInput")
        return di[name]

    x_d = inp("x", [IN, N])
    ct_d = inp("ct", [E, N])
    gdT_d = inp("gdT", [2, N, N])
    if sharded_d:
        shard_d = inp("gd_shard", [2, SHARD, N])
    emb_W_d = inp("emb_W", [IN, E])
    emb_b_d = inp("emb_b", [E])
    att_W_d = inp("att_W", [E, E])
    a12_d = inp("a12", [E, 2])
    emb2_W_d = inp("emb2_W", [N, N])
    emb2_b_d = inp("emb2_b", [N])
    lin1_W_d = inp("lin1_W", [N, N])
    lin2_W_d = inp("lin2_W", [N, N])
    lin2_b_d = inp("lin2_b", [N])
    ln_w_d = inp("ln_w", [N])
    ln_b_d = inp("ln_b", [N])
    cellGLT_d = inp("cell_GLT", [G, N])
    GLlin_W_d = inp("GLlin_W", [G, E])
    GLlin2_W_d = inp("GLlin2_W", [D2, D2])

    ht_d = nc.dram_tensor("ht", [E, N], F32, kind="ExternalOutput")
    ctn_d = nc.dram_tensor("ct_new", [E, N], F32, kind="ExternalOutput")

    with tile.TileContext(nc) as tc:
        _emit(nc, tc, di, ht_d, ctn_d, sharded_d)
    nc.compile()
    return nc


def _emit(nc, tc, di, ht_d, ctn_d, sharded_d):
    import contextlib
    ctx = contextlib.ExitStack()
    with ctx:
        persist = ctx.enter_context(tc.tile_pool(name="persist", bufs=1))
        streams = ctx.enter_context(tc.tile_pool(name="streams", bufs=3))
        work = ctx.enter_context(tc.tile_pool(name="work", bufs=2))
        small = ctx.enter_context(tc.tile_pool(name="small", bufs=4))
        mid = ctx.enter_context(tc.tile_pool(name="mid", bufs=2))
        dram = ctx.enter_context(tc.tile_pool(name="dram", bufs=1, space="DRAM"))
        pacc = ctx.enter_context(tc.tile_pool(name="pacc", bufs=1, space="PSUM"))
        psc = ctx.enter_context(tc.tile_pool(name="psc", bufs=2, space="PSUM"))
        ptr = ctx.enter_context(tc.tile_pool(name="ptr", bufs=2, space="PSUM"))

        def chunk(ap2d, c):
            return ap2d[:, c * FC:(c + 1) * FC]

        # ---------------- constants / small loads ----------------
        I128 = persist.tile([P, P], F32, tag="I128")
        make_identity(nc, I128[:])
        ones_col = persist.tile([P, 1], F32, tag="ones_col")
        nc.vector.memset(ones_col[:], 1.0)
        ones_row = persist.tile([1, P], F32, tag="ones_row")
        nc.vector.memset(ones_row[:], 1.0)

        x_sb = streams.tile([IN, N], F32, tag="stream")
        nc.sync.dma_start(out=x_sb[:], in_=di["x"][:])
        embW_sb = persist.tile([IN, E], F32, tag="embW")
        nc.sync.dma_start(out=embW_sb[:], in_=di["emb_W"][:])
        embb_sb = persist.tile([E, 1], F32, tag="embb")
        nc.gpsimd.dma_start(out=embb_sb[:], in_=_ap(di["emb_b"][:], 0, [[1, E], [0, 1]]))
        attW_sb = persist.tile([E, E], F32, tag="attW")
        nc.sync.dma_start(out=attW_sb[:], in_=di["att_W"][:])
        a12_sb = persist.tile([E, 2], F32, tag="a12")
        nc.sync.dma_start(out=a12_sb[:], in_=di["a12"][:])
        GLlinW_sb = persist.tile([G, E], F32, tag="GLlinW")
        nc.sync.dma_start(out=GLlinW_sb[:], in_=di["GLlin_W"][:])
        GLlin2_sb = persist.tile([D2, D2], F32, tag="GLlin2")
        nc.sync.dma_start(out=GLlin2_sb[:], in_=di["GLlin2_W"][:])
        e2b_sb = persist.tile([1, N], F32, tag="e2b")
        nc.gpsimd.dma_start(out=e2b_sb[:], in_=_ap(di["emb2_b"][:], 0, [[0, 1], [1, N]]))
        l2b_sb = persist.tile([1, N], F32, tag="l2b")
        nc.gpsimd.dma_start(out=l2b_sb[:], in_=_ap(di["lin2_b"][:], 0, [[0, 1], [1, N]]))
        lnw_bc = persist.tile([P, N], F32, tag="lnw_bc")
        nc.gpsimd.dma_start(out=lnw_bc[:], in_=_ap(di["ln_w"][:], 0, [[0, P], [1, N]]))
        lnb_bc = persist.tile([P, N], F32, tag="lnb_bc")
        nc.gpsimd.dma_start(out=lnb_bc[:], in_=_ap(di["ln_b"][:], 0, [[0, P], [1, N]]))
        ct_sb = persist.tile([E, N], F32, tag="ct_sb")
        nc.sync.dma_start(out=ct_sb[:], in_=di["ct"][:])

        # ---------------- d pre-pass:  d_i = (1 + sum_j graph_data[i].sum(1))^-1/2 --------
        # row sums of M are partition sums of M^T; compute via ones-matmul over
        # gdT tiles (the per-core shard), AllReduce the partials.
        nblk = SHARD // P if sharded_d else NB
        src = di["gd_shard"] if sharded_d else di["gdT"]
        ib_dr = dram.tile([2, N], F32)
        ob_dr = dram.tile([2, N], F32)
        for mat in range(2):
            psum_d = pacc.tile([1, N], F32, tag="acc")
            for blk in range(nblk):
                gtile = streams.tile([P, N], F32, tag="stream")
                nc.sync.dma_start(out=gtile[:], in_=src[mat, blk * P:(blk + 1) * P, :])
                for c in range(NCH):
                    nc.tensor.matmul(chunk(psum_d[:], c), ones_col[:],
                                     chunk(gtile[:], c),
                                     start=(blk == 0), stop=(blk == nblk - 1))
            dsum_sb = mid.tile([1, N], F32, tag="fin2k")
            nc.scalar.activation(dsum_sb[:], psum_d[:], AF.Copy)
            nc.gpsimd.dma_start(out=ib_dr[mat], in_=dsum_sb[:])
        if sharded_d:
            nc.gpsimd.collective_compute(
                "AllReduce", ALU.add,
                replica_groups=[list(range(NCORES))],
                ins=[ib_dr.opt()], outs=[ob_dr.opt()])
            dall_src = ob_dr
        else:
            dall_src = ib_dr
        dall_sb = mid.tile([2, N], F32, tag="fin2k")
        nc.sync.dma_start(out=dall_sb[:], in_=dall_src[:])
        # d = (rowsum + 1)^(-1/2)
        dp1 = mid.tile([2, N], F32, tag="fin2k")
        nc.vector.tensor_scalar_add(dp1[:], dall_sb[:], 1.0)
        drec = mid.tile([2, N], F32, tag="fin2k")
        nc.vector.reciprocal(drec[:], dp1[:])
        dvec = mid.tile([2, N], F32, tag="fin2k")
        nc.scalar.activation(dvec[:], drec[:], AF.Sqrt)
        d_dr = dram.tile([2, N], F32)
        nc.gpsimd.dma_start(out=d_dr[:], in_=dvec[:])
        # reload in consumption layouts
        d1T = persist.tile([P, NB], F32, tag="d1T")   # d1T[p, nb] = d1[nb*128+p]
        nc.gpsimd.dma_start(out=d1T[:], in_=_ap(d_dr[:], 0, [[1, P], [P, NB]]))
        d2T = persist.tile([P, NB], F32, tag="d2T")
        nc.gpsimd.dma_start(out=d2T[:], in_=_ap(d_dr[:], N, [[1, P], [P, NB]]))

        # ---------------- embed:  xeT = lrelu(emb_W^T @ x + emb_b)  [E, N] ----------
        psum_xe = pacc.tile([E, N], F32, tag="acc")
        for c in range(NCH):
            nc.tensor.matmul(chunk(psum_xe[:], c), embW_sb[:], chunk(x_sb[:], c),
                             start=True, stop=True)
        xeT = persist.tile([E, N], F32, tag="xeT")
        xe0 = mid.tile([E, N], F32, tag="fin2k")
        nc.scalar.activation(xe0[:], psum_xe[:], AF.Identity, bias=embb_sb[:])
        xet = mid.tile([E, N], F32, tag="fin2k")
        nc.vector.tensor_scalar_mul(xet[:], xe0[:], 0.01)
        nc.vector.tensor_tensor(xeT[:], xe0[:], xet[:], ALU.max)

        # ---------------- h = xe @ att_W  (feature-major hT [E, N]) ----------
        psum_h = pacc.tile([E, N], F32, tag="acc")
        for c in range(NCH):
            nc.tensor.matmul(chunk(psum_h[:], c), attW_sb[:], chunk(xeT[:], c),
                             start=True, stop=True)
        hT = persist.tile([E, N], F32, tag="hT_tb")
        nc.scalar.activation(hT[:], psum_h[:], AF.Copy)

        # u = h @ a1, v = h @ a2  -> uv [2, N]
        psum_uv = pacc.tile([2, N], F32, tag="acc")
        for c in range(NCH):
            nc.tensor.matmul(chunk(psum_uv[:], c), a12_sb[:], chunk(hT[:], c),
                             start=True, stop=True)
        uv_sb = mid.tile([2, N], F32, tag="fin2k")
        nc.scalar.activation(uv_sb[:], psum_uv[:], AF.Copy)
        uv_dr = dram.tile([2, N], F32)
        nc.gpsimd.dma_start(out=uv_dr[:], in_=uv_sb[:])
        u_bc = persist.tile([P, N], F32, tag="u_gate")   # u broadcast across partitions
        nc.gpsimd.dma_start(out=u_bc[:], in_=_ap(uv_dr[:], 0, [[0, P], [1, N]]))
        vT = persist.tile([P, NB], F32, tag="vT")      # vT[p, jb] = v[jb*128+p]
        nc.gpsimd.dma_start(out=vT[:], in_=_ap(uv_dr[:], N, [[1, P], [P, NB]]))
        vT001 = persist.tile([P, NB], F32, tag="vT001")
        nc.vector.tensor_scalar_mul(vT001[:], vT[:], 0.01)

        # h1_im[j, :, e] = h[j, e] (i-major h tiles), col 64 = 1 (softmax denom trick)
        h1_im = persist.tile([P, NB, E + 1], F32, tag="h1_im")
        nc.vector.memset(h1_im[:], 1.0)
        for jb in range(NB):
            tp = ptr.tile([P, E], F32, tag="ptr")
            nc.tensor.transpose(tp[:], hT[:, jb * P:(jb + 1) * P], I128[0:E, 0:E])
            nc.vector.tensor_copy(h1_im[:, jb, 0:E], tp[:])

        # ---------------- attention:  A = relu(softmax_j(lrelu(u_i+v_j)) @ h) ------
        # P_jb[j, i] = exp(lrelu(u_i + v_j)); accumulate [h|1]^T @ P -> [E+1, N]
        psum_A = pacc.tile([E + 1, N], F32, tag="acc")
        for jb in range(NB):
            lr = work.tile([P, N], F32, tag="big_a")
            nc.scalar.activation(lr[:], u_bc[:], AF.Exp, bias=vT[:, jb:jb + 1])
            lr2 = work.tile([P, N], F32, tag="big_a")
            nc.scalar.activation(lr2[:], u_bc[:], AF.Exp, scale=0.01,
                                 bias=vT001[:, jb:jb + 1])
            nc.vector.tensor_tensor(lr[:], lr[:], lr2[:], ALU.max)
            for c in range(NCH):
                nc.tensor.matmul(chunk(psum_A[:], c), h1_im[:, jb, :], chunk(lr[:], c),
                                 start=(jb == 0), stop=(jb == NB - 1))
        Afm = persist.tile([E + 1, N], F32, tag="y_fm")
        nc.scalar.activation(Afm[:], psum_A[:], AF.Copy)
        # transpose + normalize + relu -> A_im [i, e] tiles
        A_im = persist.tile([P, NB, E], F32, tag="A_ge")
        for ib in range(NB):
            tp = ptr.tile([P, E + 1], F32, tag="ptr")
            nc.tensor.transpose(tp[:], Afm[:, ib * P:(ib + 1) * P], I128[0:E + 1, 0:E + 1])
            rS = small.tile([P, 1], F32, tag="rS")
            nc.vector.reciprocal(rS[:], tp[:, E:E + 1])
            nc.scalar.activation(A_im[:, ib, :], tp[:, 0:E], AF.Relu, scale=rS[:])

        # ---------------- z1 = A@lin1_W ; z2 = A@lin2_W + lin2_b ; gl1 = A@emb2_W + emb2_b
        z12_fm = persist.tile([D2, N], F32, tag="z12_fm")
        cat_fm = persist.tile([D2, N], F32, tag="cat_d2bc")

        def a_matmul(w_dram, out_slice, bias_row, evict_engine="act"):
            psum_z = pacc.tile([E, N], F32, tag="acc")
            for ib in range(NB):
                w = streams.tile([P, N], F32, tag="stream")
                nc.sync.dma_start(out=w[:], in_=w_dram[ib * P:(ib + 1) * P, :])
                last = (ib == NB - 1) and bias_row is None
                for c in range(NCH):
                    nc.tensor.matmul(chunk(psum_z[:], c), A_im[:, ib, :], chunk(w[:], c),
                                     start=(ib == 0), stop=last)
            if bias_row is not None:
                for c in range(NCH):
                    nc.tensor.matmul(chunk(psum_z[:], c), ones_row[0:1, 0:E],
                                     chunk(bias_row, c), start=False, stop=True)
            if evict_engine == "act":
                nc.scalar.activation(out_slice, psum_z[:], AF.Copy)
            else:
                nc.vector.tensor_copy(out_slice, psum_z[:])

        a_matmul(di["lin1_W"][:], z12_fm[0:E, :], None)
        a_matmul(di["lin2_W"][:], z12_fm[E:D2, :], l2b_sb[:], "vec")
        a_matmul(di["emb2_W"][:], cat_fm[0:E, :], e2b_sb[:])
        # gl2 = (cell_GL @ GLlin_W)^T = GLlin_W^T @ cell_GL^T
        psum_g2 = pacc.tile([E, N], F32, tag="acc")
        cgl = streams.tile([G, N], F32, tag="stream")
        nc.sync.dma_start(out=cgl[:], in_=di["cell_GLT"][:])
        for c in range(NCH):
            nc.tensor.matmul(chunk(psum_g2[:], c), GLlinW_sb[:], chunk(cgl[:], c),
                             start=True, stop=True)
        nc.vector.tensor_copy(cat_fm[E:D2, :], psum_g2[:])

        # z12 i-major tiles
        z12_im = persist.tile([P, NB, D2], F32, tag="z12_im")
        for ib in range(NB):
            tp = ptr.tile([P, P], F32, tag="ptr")
            nc.tensor.transpose(tp[:], z12_fm[:, ib * P:(ib + 1) * P], I128[:])
            nc.vector.tensor_copy(z12_im[:, ib, :], tp[:])

        # ---------------- branch chain part 1:  Y12 = (z12 * d1[n]) @ Mt1, * d1[m]
        def l_matmul(mat_idx, lhs_im, dT, psum_tag):
            psum_y = pacc.tile([D2, N], F32, tag=psum_tag)
            for nb in range(NB):
                mt = streams.tile([P, N], F32, tag="stream")
                nc.sync.dma_start(out=mt[:], in_=di["gdT"][mat_idx, nb * P:(nb + 1) * P, :])
                nc.vector.tensor_tensor(mt[:, nb * P:(nb + 1) * P],
                                        mt[:, nb * P:(nb + 1) * P], I128[:], ALU.add)
                zd = small.tile([P, D2], F32, tag="zd")
                nc.vector.tensor_scalar_mul(zd[:], lhs_im[:, nb, :], dT[:, nb:nb + 1])
                for c in range(NCH):
                    nc.tensor.matmul(chunk(psum_y[:], c), zd[:], chunk(mt[:], c),
                                     start=(nb == 0), stop=(nb == NB - 1))
            return psum_y

        psum_y = l_matmul(0, z12_im, d1T, "acc")
        y_fm = persist.tile([D2, N], F32, tag="y_fm")
        nc.scalar.activation(y_fm[:], psum_y[:], AF.Copy)
        u12_im = persist.tile([P, NB, D2], F32, tag="u12_zz12")
        for mb in range(NB):
            tp = ptr.tile([P, P], F32, tag="ptr")
            nc.tensor.transpose(tp[:], y_fm[:, mb * P:(mb + 1) * P], I128[:])
            nc.scalar.activation(u12_im[:, mb, :], tp[:], AF.Copy, scale=d1T[:, mb:mb + 1])

        # zz1 = u1 @ lin1_W ; zz2 = u2 @ lin2_W + lin2_b   (into one psum, row halves)
        psum_zz = pacc.tile([D2, N], F32, tag="acc")
        for ib in range(NB):
            w = streams.tile([P, N], F32, tag="stream")
            nc.sync.dma_start(out=w[:], in_=di["lin1_W"][ib * P:(ib + 1) * P, :])
            for c in range(NCH):
                nc.tensor.matmul(chunk(psum_zz[0:E, :], c), u12_im[:, ib, 0:E],
                                 chunk(w[:], c), start=(ib == 0), stop=(ib == NB - 1))
        for ib in range(NB):
            w = streams.tile([P, N], F32, tag="stream")
            nc.sync.dma_start(out=w[:], in_=di["lin2_W"][ib * P:(ib + 1) * P, :])
            for c in range(NCH):
                nc.tensor.matmul(chunk(psum_zz[E:D2, :], c), u12_im[:, ib, E:D2],
                                 chunk(w[:], c), start=(ib == 0), stop=False)
        for c in range(NCH):
            nc.tensor.matmul(chunk(psum_zz[E:D2, :], c), ones_row[0:1, 0:E],
                             chunk(l2b_sb[:], c), start=False, stop=True)
        zz_fm = persist.tile([D2, N], F32, tag="y_fm")
        nc.scalar.activation(zz_fm[:], psum_zz[:], AF.Copy)
        zz12_im = persist.tile([P, NB, D2], F32, tag="u12_zz12")
        for mb in range(NB):
            tp = ptr.tile([P, P], F32, tag="ptr")
            nc.tensor.transpose(tp[:], zz_fm[:, mb * P:(mb + 1) * P], I128[:])
            nc.vector.tensor_copy(zz12_im[:, mb, :], tp[:])

        # TB = (zz12 * d2[n]) @ Mt2   (the trailing * d2[m] is applied at combine)
        psum_tb = l_matmul(1, zz12_im, d2T, "acc")
        tb_fm = persist.tile([D2, N], F32, tag="hT_tb")
        nc.scalar.activation(tb_fm[:], psum_tb[:], AF.Copy)

        # ---------------- graph-learn:  ge^T = GLlin2_W^T @ [gl1; gl2] ----------
        psum_ge = pacc.tile([D2, N], F32, tag="acc")
        for c in range(NCH):
            nc.tensor.matmul(chunk(psum_ge[:], c), GLlin2_sb[:], chunk(cat_fm[:], c),
                             start=True, stop=True)
        ge_sb = persist.tile([D2, N], F32, tag="A_ge")
        nc.scalar.activation(ge_sb[:], psum_ge[:], AF.Copy)

        # graph_learn = I + softmax_m(relu(ge ge^T)); t1 = z12 @ graph_learn
        # computed as: P'[n,m] = exp(relu(s)-mx'), fold 1/rowsum into lhsT, +I via z12_fm
        psum_t1 = pacc.tile([D2, N], F32, tag="acc")
        for nb in range(NB):
            # pass 1: row maxes only (scores recomputed in pass 2 — PSUM is the
            # scarce resource here, PE has headroom)
            mx4 = small.tile([P, NCH], F32, tag="mx4")
            for c in range(NCH):
                s_c = psc.tile([P, FC], F32, tag="psc")
                nc.tensor.matmul(s_c[:], ge_sb[:, nb * P:(nb + 1) * P], chunk(ge_sb[:], c),
                                 start=True, stop=True)
                nc.vector.reduce_max(mx4[:, c:c + 1], s_c[:], axis=AX.X)
            mx = small.tile([P, 1], F32, tag="mx")
            nc.vector.reduce_max(mx[:], mx4[:], axis=AX.X)
            nc.vector.tensor_scalar_max(mx[:], mx[:], 0.0)      # relu'd max
            nmx = small.tile([P, 1], F32, tag="nmx")
            nc.vector.tensor_scalar_mul(nmx[:], mx[:], -1.0)
            eneg = small.tile([P, 1], F32, tag="eneg")
            nc.scalar.activation(eneg[:], nmx[:], AF.Exp)
            pn = work.tile([P, N], F32, tag="big_a")
            ssum4 = small.tile([P, NCH], F32, tag="ssum4")
            for c in range(NCH):
                s_c = psc.tile([P, FC], F32, tag="psc")
                nc.tensor.matmul(s_c[:], ge_sb[:, nb * P:(nb + 1) * P], chunk(ge_sb[:], c),
                                 start=True, stop=True)
                nc.scalar.activation(chunk(pn[:], c), s_c[:], AF.Exp, bias=nmx[:])
                nc.vector.tensor_scalar(chunk(pn[:], c), chunk(pn[:], c), eneg[:], None,
                                        op0=ALU.max, op1=ALU.add,
                                        accum_out=ssum4[:, c:c + 1])
            S = small.tile([P, 1], F32, tag="S")
            nc.vector.reduce_sum(S[:], ssum4[:], axis=AX.X)
            rr = small.tile([P, 1], F32, tag="rr")
            nc.vector.reciprocal(rr[:], S[:])
            zgl = small.tile([P, D2], F32, tag="zd")
            nc.vector.tensor_scalar_mul(zgl[:], z12_im[:, nb, :], rr[:])
            for c in range(NCH):
                nc.tensor.matmul(chunk(psum_t1[:], c), zgl[:], chunk(pn[:], c),
                                 start=(nb == 0), stop=(nb == NB - 1))

        # ---------------- combine + layernorm + gates ----------------
        d2bc = persist.tile([P, N], F32, tag="cat_d2bc")  # d2bc[p, m] = d2[m]
        nc.gpsimd.dma_start(out=d2bc[:], in_=_ap(d_dr[:], N, [[0, P], [1, N]]))
        t_fm = work.tile([D2, N], F32, tag="big_a")
        nc.vector.tensor_tensor(t_fm[:], tb_fm[:], d2bc[:], ALU.mult)
        nc.vector.tensor_tensor(t_fm[:], t_fm[:], z12_fm[:], ALU.add)
        nc.vector.tensor_tensor(t_fm[:], t_fm[:], psum_t1[:], ALU.add)

        bnst = small.tile([P, NCH, 6], F32, tag="bnst")
        for c in range(NCH):
            nc.vector.bn_stats(bnst[:, c, :], chunk(t_fm[:], c))
        mv = small.tile([P, 2], F32, tag="mv")
        nc.vector.bn_aggr(mv[:], bnst[:])
        veps = small.tile([P, 1], F32, tag="veps")
        nc.vector.tensor_scalar_add(veps[:], mv[:, 1:2], 1e-5)
        rv = small.tile([P, 1], F32, tag="rv")
        nc.vector.reciprocal(rv[:], veps[:])
        rstd = small.tile([P, 1], F32, tag="rstd")
        nc.scalar.activation(rstd[:], rv[:], AF.Sqrt)
        ln_out = work.tile([D2, N], F32, tag="big_a")
        nc.vector.tensor_scalar(ln_out[:], t_fm[:], mv[:, 0:1], rstd[:],
                                op0=ALU.subtract, op1=ALU.mult)
        nc.vector.tensor_tensor(ln_out[:], ln_out[:], lnw_bc[:], ALU.mult)
        nc.vector.tensor_tensor(ln_out[:], ln_out[:], lnb_bc[:], ALU.add)

        gate = persist.tile([E, N], F32, tag="u_gate")
        nc.scalar.activation(gate[:], ln_out[E:D2, :], AF.Gelu)
        x_new = ln_out[0:E, :]

        # ct_new = x_new + gate * (ct - x_new)
        fin1 = work.tile([E, N], F32, tag="fin", bufs=3)
        nc.vector.tensor_tensor(fin1[:], ct_sb[:], x_new, ALU.subtract)
        nc.vector.tensor_tensor(fin1[:], fin1[:], gate[:], ALU.mult)
        ctn_sb = work.tile([E, N], F32, tag="fin", bufs=3)
        nc.vector.tensor_tensor(ctn_sb[:], fin1[:], x_new, ALU.add)
        nc.sync.dma_start(out=ctn_d[:], in_=ctn_sb[:])

        # elu(ct_new) = max(ct_new, exp(min(ct_new, 0)) - 1)
        ex = work.tile([E, N], F32, tag="fin", bufs=3)
        nc.vector.tensor_scalar_min(ex[:], ctn_sb[:], 0.0)
        nc.scalar.activation(ex[:], ex[:], AF.Exp)
        nc.vector.tensor_scalar_add(ex[:], ex[:], -1.0)
        el = work.tile([E, N], F32, tag="fin", bufs=3)
        nc.vector.tensor_tensor(el[:], ex[:], ctn_sb[:], ALU.max)

        # ht = xT + gate * (elu - xT)
        nc.vector.tensor_tensor(el[:], el[:], xeT[:], ALU.subtract)
        nc.vector.tensor_tensor(el[:], el[:], gate[:], ALU.mult)
        ht_sb = work.tile([E, N], F32, tag="fin", bufs=3)
        nc.vector.tensor_tensor(ht_sb[:], el[:], xeT[:], ALU.add)
        nc.sync.dma_start(out=ht_d[:], in_=ht_sb[:])


def host_prep(inputs):
    """Shared (batch-independent) host-side layout prep."""
    gdT = np.ascontiguousarray(np.swapaxes(np.asarray(inputs["graph_data"]), 1, 2))
    att_a = np.asarray(inputs["att_a"])
    a12 = np.ascontiguousarray(np.stack([att_a[:E, 0], att_a[E:, 0]], axis=1))
    cell_GLT = np.ascontiguousarray(np.asarray(inputs["cell_GL"]).T)
    common = {
        "gdT": gdT,
        "emb_W": np.asarray(inputs["emb_W"]),
        "emb_b": np.asarray(inputs["emb_b"]),
        "att_W": np.asarray(inputs["att_W"]),
        "a12": a12,
        "emb2_W": np.asarray(inputs["emb2_W"]),
        "emb2_b": np.asarray(inputs["emb2_b"]),
        "lin1_W": np.asarray(inputs["lin1_W"]),
        "lin2_W": np.asarray(inputs["lin2_W"]),
        "lin2_b": np.asarray(inputs["lin2_b"]),
        "ln_w": np.asarray(inputs["ln_w"]),
        "ln_b": np.asarray(inputs["ln_b"]),
        "cell_GLT": cell_GLT,
        "GLlin_W": np.asarray(inputs["GLlin_W"]),
        "GLlin2_W": np.asarray(inputs["GLlin2_W"]),
    }
    return common


def make_in_maps(inputs, sharded_d=True):
    common = host_prep(inputs)
    x = np.asarray(inputs["x"])
    ct = np.asarray(inputs["ct"])
    in_maps = []
    for b in range(NCORES):
        m = dict(common)
        m["x"] = np.ascontiguousarray(x[b])
        m["ct"] = np.ascontiguousarray(ct[b])
        if sharded_d:
            m["gd_shard"] = np.ascontiguousarray(
                common["gdT"][:, b * SHARD:(b + 1) * SHARD, :])
        in_maps.append(m)
    return in_maps


def kernel(**inputs):
    if "nc" not in _CACHE:
        _CACHE["nc"] = build_program(sharded_d=True)
    nc = _CACHE["nc"]
    in_maps = make_in_maps(inputs, sharded_d=True)
    res = run_bass_kernel_spmd(nc, in_maps, list(range(NCORES)))
    ht = np.stack([res.results[b]["ht"] for b in range(NCORES)])
    ct_new = np.stack([res.results[b]["ct_new"] for b in range(NCORES)])
    return ht, ct_new
